# revision 17
# baseline (speedup 1.0000x reference)
"""Trainium2 Bass kernel for nn_CrossCategoryLoss.

loss(row) = sum_t relu(log_a[A_t] + log_b[B_t] - c_t)
  with c_t = log_g[G_t] (pos) or log(1 - exp(log_g[G_t])) (not).

Math (per row; S = lsg - lsa - lsb):
  pos term: relu(alpha[a] + beta[b] - gamma[g] + S)
  not term: relu(alpha[a] + beta[b] - ln(sum_g - exp(gamma[g])) + S)

Engine strategy (v2): DVE was the bottleneck at ~46.5k cycles/tile in the
fp32-stage-1 version. This version runs the whole post-exp pipeline in
fp16 (DVE 2x_1p mode needs packed 2-byte innermost APs):
  - exps are written k-major ([P, 8, R]) by ACT (strided out AP is free on
    ACT), so every per-feature slice is packed -> 2x.
  - alpha/beta/gamma are reconstructed k-major via Ln(exp) on ACT (ACT has
    slack; this buys 2x on the 14 pair adds and the q-prep).
  - softmax denominators via fp16 tree adds (2x) instead of reduce (1x).
  - a/b exps scaled by e^-ln8 (free: ACT bias) so sab = sa*sb/64 stays in
    fp16 range; the ln64 offset cancels against the pair offset exactly:
    pairs use a8+b8 = alpha+beta-ln64, and Sh = lsg - Ln(sab') = S + ln64.
  - relu via tensor_scalar max (4x mode).
  - Pool engine (gpsimd) takes the small per-row chain + some pair adds.

Sharding: pure data-parallel over 8 cores; each core handles B/8 rows.
"""

import math

import numpy as np

import concourse.bass as bass
import concourse.bacc as bacc
import concourse.mybir as mybir
from concourse.tile import TileContext
from concourse.bass_utils import run_bass_kernel_spmd

N_CORES = 8
B = 4194304
B_CORE = B // N_CORES          # 524288 rows per core
P = 128                        # partitions
ROWS_PER_PART = B_CORE // P    # 4096
R = 512                        # rows per partition per tile
LN8 = float(math.log(8.0))

F32 = mybir.dt.float32
F16 = mybir.dt.float16
AX = mybir.AxisListType
AF = mybir.ActivationFunctionType
OP = mybir.AluOpType

# (alpha_idx, beta_idx, gamma_idx, is_not) - 36 constraint terms.
_TERMS = [
    (0, 4, 4, 0), (0, 4, 1, 1), (0, 4, 2, 1),
    (0, 6, 4, 0), (0, 6, 1, 1), (0, 6, 2, 1),
    (1, 5, 5, 0), (1, 5, 0, 1), (1, 5, 2, 1),
    (1, 6, 5, 0), (1, 6, 0, 1), (1, 6, 2, 1),
    (2, 4, 4, 0), (2, 4, 1, 1), (2, 4, 2, 1),
    (2, 5, 5, 0), (2, 5, 0, 1), (2, 5, 2, 1),
    (2, 6, 6, 0), (2, 7, 7, 0), (2, 7, 2, 1),
    (4, 0, 4, 0), (4, 0, 1, 1), (4, 0, 2, 1),
    (4, 2, 4, 0), (4, 2, 1, 1), (4, 2, 2, 1),
    (5, 1, 5, 0), (5, 1, 0, 1), (5, 1, 2, 1),
    (5, 2, 5, 0), (5, 2, 0, 1), (5, 2, 2, 1),
    (6, 2, 6, 0), (7, 2, 7, 0), (7, 2, 2, 1),
]

# Pair slots in the p16 tile, ordered so every q-group's pair set is a
# contiguous slot range (enables one batched subtract per q-group):
_PAIR_SLOTS = [
    (0, 4), (0, 6), (2, 4), (4, 0), (4, 2),      # X1: q-triple {g4, w1, w2}
    (1, 5), (1, 6), (2, 5), (5, 1), (5, 2),      # X2: q-triple {g5, w0, w2}
    (2, 7), (7, 2), (2, 6), (6, 2),              # tail
]
# (q_key, pair_slot_range, d_slot_start): D[d0:d0+n] = P[p0:p1] - q
# q_key: (gamma_idx, is_not)
_QGROUPS = [
    ((4, 0), 0, 5, 0),     # g4:  D[0:5]   = P[0:5]  - q_g4
    ((1, 1), 0, 5, 5),     # w1:  D[5:10]  = P[0:5]  - q_w1
    ((2, 1), 0, 12, 10),   # w2:  D[10:22] = P[0:12] - q_w2
    ((5, 0), 5, 10, 22),   # g5:  D[22:27] = P[5:10] - q_g5
    ((0, 1), 5, 10, 27),   # w0:  D[27:32] = P[5:10] - q_w0
    ((7, 0), 10, 12, 32),  # g7:  D[32:34] = P[10:12]- q_g7
    ((6, 0), 12, 14, 34),  # g6:  D[34:36] = P[12:14]- q_g6
]
# Emission order of the d36 sub groups: the ACT-relu'd slot range (tail
# slots) first, so ACT can start its relu share while DVE finishes the rest.
_QGROUP_ORDER = [3, 4, 5, 6, 0, 1, 2]

# Engine-balance knobs (tuned against the CoreSim/HW traces):
_POOL_PAIRS = 2        # pair adds on Pool instead of DVE
_POOL_GTREE = True     # g exp-sum tree on Pool
_POOL_QP = True        # qp subtract on Pool
_POOL_TAIL = True      # final (fp32) sum-tree level on Pool
_ACT_RELU_SLOTS = 12   # d36 slots relu'd by ACT (rest on DVE 4x)


def _bcast_mid(ap, n):
    """[P, R] access pattern -> [P, n, R] with a zero-stride middle dim."""
    a = ap[:, :]
    return bass.AP(tensor=a.tensor, offset=a.offset,
                   ap=[a.ap[0], [0, n], a.ap[1]])


def _swap_free(ap):
    """View a [P, K, R] tile iterated as [P, R, K] (same memory)."""
    a = ap[:, :, :]
    return bass.AP(tensor=a.tensor, offset=a.offset,
                   ap=[a.ap[0], a.ap[2], a.ap[1]])


def build_kernel(reps: int = 1, rows_per_part: int = ROWS_PER_PART) -> bass.Bass:
    nc = bacc.Bacc("TRN2", target_bir_lowering=False, debug=False,
                   num_devices=N_CORES)

    # Restrict the ACT table chooser to the one set holding Exp+Ln (avoids
    # ~2.7us table reloads between Exp and Ln ops).
    _orig_tables = bacc.get_activation_tables

    def _one_set(arch):
        return {
            name: (fns if name == "natural_log_exp_and_others" else set())
            for name, fns in _orig_tables(arch).items()
        }

    # Register the exp-bias constant (-ln8) as a per-partition const AP the
    # same way Bass.__init__ registers 0.0/1.0 (activation converts float
    # biases to const APs and asserts one exists).
    t = nc.alloc_sbuf_tensor("const-f32-negln8", [128, 1], F32)
    nc.gpsimd.memset(t.ap(), -LN8)
    nc.const_aps.aps[(F32, -LN8)] = t.ap()
    nc.all_engine_barrier()

    bacc.get_activation_tables = _one_set
    try:
        return _build_body(nc, reps, rows_per_part)
    finally:
        bacc.get_activation_tables = _orig_tables


def _build_body(nc, reps: int, rows_per_part: int) -> bass.Bass:
    b_core = P * rows_per_part
    n_tiles = rows_per_part // R

    a_d = nc.dram_tensor("alpha_logits", [b_core, 8], F32, kind="ExternalInput")
    b_d = nc.dram_tensor("beta_logits", [b_core, 8], F32, kind="ExternalInput")
    g_d = nc.dram_tensor("gamma_logits", [b_core, 8], F32, kind="ExternalInput")
    o_d = nc.dram_tensor("loss", [b_core], F32, kind="ExternalOutput")

    a_v = a_d[:].rearrange("(p n) k -> p n k", p=P)
    b_v = b_d[:].rearrange("(p n) k -> p n k", p=P)
    g_v = g_d[:].rearrange("(p n) k -> p n k", p=P)
    o_v = o_d[:].rearrange("(p n) -> p n", p=P)

    with TileContext(nc) as tc:
        import contextlib
        rep_loop = tc.For_i(0, reps, 1) if reps > 1 else contextlib.nullcontext()
        with (
            rep_loop,
            tc.tile_pool(name="io", bufs=2) as io,
            tc.tile_pool(name="e8", bufs=1) as e8p,
            tc.tile_pool(name="work", bufs=1) as work,
            tc.tile_pool(name="p16p", bufs=2) as p16p,
            tc.tile_pool(name="big", bufs=1) as big,
            tc.tile_pool(name="outp", bufs=1) as outp,
        ):
            for j in range(n_tiles):
                sl = slice(j * R, (j + 1) * R)

                a_t = io.tile([P, R, 8], F32, tag="a")
                b_t = io.tile([P, R, 8], F32, tag="b")
                g_t = io.tile([P, R, 8], F32, tag="g")
                nc.sync.dma_start(out=a_t, in_=a_v[:, sl, :])
                nc.sync.dma_start(out=b_t, in_=b_v[:, sl, :])
                nc.sync.dma_start(out=g_t, in_=g_v[:, sl, :])

                # --- exps, k-major fp16 [P, 8, R] (strided out is free on
                # ACT). a/b exps carry bias -ln8 so sab = sa*sb/64 stays in
                # fp16 range; lns of these tiles give alpha-ln8 / beta-ln8.
                ea8 = e8p.tile([P, 8, R], F16, tag="ea8")
                eb8 = e8p.tile([P, 8, R], F16, tag="eb8")
                eg8 = e8p.tile([P, 8, R], F16, tag="eg8")
                nc.scalar.activation(out=_swap_free(ea8), in_=a_t,
                                     func=AF.Exp, bias=-LN8)
                nc.scalar.activation(out=_swap_free(eb8), in_=b_t,
                                     func=AF.Exp, bias=-LN8)
                nc.scalar.activation(out=_swap_free(eg8), in_=g_t,
                                     func=AF.Exp)

                # --- softmax denominators via fp16 tree adds (2x).
                # s4/s2 scratch is shared across the three tensors (the tree
                # ops serialize on DVE anyway); s1 sums must coexist.
                # a/b trees share scratch (both DVE, serialized anyway); the
                # g tree can run on Pool in parallel so it gets its own.
                sums = {}
                for name, e_t in (("a", ea8), ("b", eb8), ("g", eg8)):
                    on_pool = _POOL_GTREE and name == "g"
                    eng = nc.gpsimd if on_pool else nc.vector
                    sfx = "g" if on_pool else "ab"
                    s4 = work.tile([P, 4, R], F16, tag=f"s4{sfx}")
                    eng.tensor_add(s4, e_t[:, 0:4, :], e_t[:, 4:8, :])
                    eng.tensor_add(s4[:, 0:2, :], s4[:, 0:2, :], s4[:, 2:4, :])
                    s1 = work.tile([P, R], F16, tag=f"s1{name}")
                    eng.tensor_add(s1, s4[:, 0, :], s4[:, 1, :])
                    sums[name] = s1
                sg_t = sums["g"]

                # --- k-major logits via Ln(exp) on ACT (enables 2x pairs).
                # a8 = alpha - ln8 (in place over ea8), b8 = beta - ln8,
                # gq = gamma[4:8] (in place over eg8[4:8], dead after g-tree).
                a8, b8 = ea8, eb8
                nc.scalar.activation(out=a8, in_=ea8, func=AF.Ln)
                nc.scalar.activation(out=b8, in_=eb8, func=AF.Ln)
                gq = eg8[:, 4:8, :]
                nc.scalar.activation(out=gq, in_=eg8[:, 4:8, :], func=AF.Ln)

                # --- Sh = lsg - Ln(sab') = S + ln64 (cancels pair offset) ---
                sab = work.tile([P, R], F16, tag="sab")
                nc.gpsimd.tensor_mul(sab, sums["a"], sums["b"])
                lsab = work.tile([P, R], F16, tag="lsab")
                nc.scalar.activation(out=lsab, in_=sab, func=AF.Ln)
                lsg = work.tile([P, R], F16, tag="lsg")
                nc.scalar.activation(out=lsg, in_=sg_t, func=AF.Ln)
                sh_t = work.tile([P, R], F16, tag="Sh")
                nc.gpsimd.tensor_sub(sh_t, lsg, lsab)

                # --- q tiles: qp[k] = gamma[4+k] - Sh (pos),
                #              qn[k] = ln(sg - eg[k]) - Sh (not, k in 0..2)
                # wl and qn run in place over wp; qp in place over gq.
                wp = work.tile([P, 3, R], F16, tag="wp")
                nc.gpsimd.tensor_sub(wp, _bcast_mid(sg_t, 3), eg8[:, 0:3, :])
                nc.scalar.activation(out=wp, in_=wp, func=AF.Ln)
                qn = wp
                nc.gpsimd.tensor_sub(qn, wp, _bcast_mid(sh_t, 3))

                # --- 14 pair sums (fp16 2x); emitted before qp so DVE does
                # not stall waiting for the Pool/ACT Sh chain. A few go to
                # Pool for balance.
                p16 = p16p.tile([P, 14, R], F16, tag="p16")
                for i, (ai, bi) in enumerate(_PAIR_SLOTS):
                    eng = nc.gpsimd if i < _POOL_PAIRS else nc.vector
                    eng.tensor_add(p16[:, i, :], a8[:, ai, :], b8[:, bi, :])

                qp = gq
                qp_eng = nc.gpsimd if _POOL_QP else nc.vector
                qp_eng.tensor_sub(qp, gq, _bcast_mid(sh_t, 4))
                q = {(4 + k, 0): qp[:, k, :] for k in range(4)}
                q.update({(k, 1): qn[:, k, :] for k in range(3)})

                # --- d36 = p - q (7 batched subs, 2x), relu (split DVE 4x /
                # ACT, they run on disjoint slot ranges), tree sum ---
                d36 = big.tile([P, 36, R], F16, tag="d36")
                for gi in _QGROUP_ORDER:
                    qkey, p0, p1, d0 = _QGROUPS[gi]
                    n = p1 - p0
                    nc.vector.tensor_sub(
                        d36[:, d0:d0 + n, :], p16[:, p0:p1, :],
                        _bcast_mid(q[qkey], n),
                    )
                ns = 36 - _ACT_RELU_SLOTS
                if _ACT_RELU_SLOTS:
                    nc.scalar.activation(out=d36[:, ns:36, :],
                                         in_=d36[:, ns:36, :], func=AF.Relu)
                nc.vector.tensor_scalar_max(out=d36[:, 0:ns, :],
                                            in0=d36[:, 0:ns, :], scalar1=0.0)

                nc.vector.tensor_add(d36[:, 0:18, :], d36[:, 0:18, :],
                                     d36[:, 18:36, :])
                nc.vector.tensor_add(d36[:, 0:9, :], d36[:, 0:9, :],
                                     d36[:, 9:18, :])
                nc.vector.tensor_add(d36[:, 0, :], d36[:, 0, :], d36[:, 8, :])
                nc.vector.tensor_add(d36[:, 0:4, :], d36[:, 0:4, :],
                                     d36[:, 4:8, :])
                nc.vector.tensor_add(d36[:, 0:2, :], d36[:, 0:2, :],
                                     d36[:, 2:4, :])
                loss_t = outp.tile([P, R], F32, tag="loss")
                tail_eng = nc.gpsimd if _POOL_TAIL else nc.vector
                tail_eng.tensor_add(loss_t, d36[:, 0, :], d36[:, 1, :])
                nc.sync.dma_start(out=o_v[:, sl], in_=loss_t)

    nc.compile()
    return nc


_NC_CACHE = None


def _get_nc():
    global _NC_CACHE
    if _NC_CACHE is None:
        _NC_CACHE = build_kernel()
    return _NC_CACHE


def kernel(alpha_logits, beta_logits, gamma_logits, _trace=False):
    nc = _get_nc()
    in_maps = []
    for c in range(N_CORES):
        sl = slice(c * B_CORE, (c + 1) * B_CORE)
        in_maps.append({
            "alpha_logits": np.ascontiguousarray(alpha_logits[sl]),
            "beta_logits": np.ascontiguousarray(beta_logits[sl]),
            "gamma_logits": np.ascontiguousarray(gamma_logits[sl]),
        })
    res = run_bass_kernel_spmd(nc, in_maps, core_ids=list(range(N_CORES)),
                               trace=_trace)
    out = np.concatenate([r["loss"] for r in res.results])
    if _trace:
        kernel.last_result = res
    return out


# revision 19
# speedup vs baseline: 1.1566x; 1.1566x over previous
"""Trainium2 Bass kernel for nn_CrossCategoryLoss.

loss(row) = sum_t relu(log_a[A_t] + log_b[B_t] - c_t)
  with c_t = log_g[G_t] (pos) or log(1 - exp(log_g[G_t])) (not).

Math (per row; S = lsg - lsa - lsb):
  pos term: relu(alpha[a] + beta[b] - gamma[g] + S)
  not term: relu(alpha[a] + beta[b] - ln(sum_g - exp(gamma[g])) + S)

Engine strategy (v2): DVE was the bottleneck at ~46.5k cycles/tile in the
fp32-stage-1 version. This version runs the whole post-exp pipeline in
fp16 (DVE 2x_1p mode needs packed 2-byte innermost APs):
  - exps are written k-major ([P, 8, R]) by ACT (strided out AP is free on
    ACT), so every per-feature slice is packed -> 2x.
  - alpha/beta/gamma are reconstructed k-major via Ln(exp) on ACT (ACT has
    slack; this buys 2x on the 14 pair adds and the q-prep).
  - softmax denominators via fp16 tree adds (2x) instead of reduce (1x).
  - a/b exps scaled by e^-ln8 (free: ACT bias) so sab = sa*sb/64 stays in
    fp16 range; the ln64 offset cancels against the pair offset exactly:
    pairs use a8+b8 = alpha+beta-ln64, and Sh = lsg - Ln(sab') = S + ln64.
  - relu via tensor_scalar max (4x mode).
  - Pool engine (gpsimd) takes the small per-row chain + some pair adds.

Sharding: pure data-parallel over 8 cores; each core handles B/8 rows.
"""

import math

import numpy as np

import concourse.bass as bass
import concourse.bacc as bacc
import concourse.mybir as mybir
from concourse.tile import TileContext
from concourse.bass_utils import run_bass_kernel_spmd

N_CORES = 8
B = 4194304
B_CORE = B // N_CORES          # 524288 rows per core
P = 128                        # partitions
ROWS_PER_PART = B_CORE // P    # 4096
R = 512                        # rows per partition per tile
LN8 = float(math.log(8.0))

F32 = mybir.dt.float32
F16 = mybir.dt.float16
AX = mybir.AxisListType
AF = mybir.ActivationFunctionType
OP = mybir.AluOpType

# (alpha_idx, beta_idx, gamma_idx, is_not) - 36 constraint terms.
_TERMS = [
    (0, 4, 4, 0), (0, 4, 1, 1), (0, 4, 2, 1),
    (0, 6, 4, 0), (0, 6, 1, 1), (0, 6, 2, 1),
    (1, 5, 5, 0), (1, 5, 0, 1), (1, 5, 2, 1),
    (1, 6, 5, 0), (1, 6, 0, 1), (1, 6, 2, 1),
    (2, 4, 4, 0), (2, 4, 1, 1), (2, 4, 2, 1),
    (2, 5, 5, 0), (2, 5, 0, 1), (2, 5, 2, 1),
    (2, 6, 6, 0), (2, 7, 7, 0), (2, 7, 2, 1),
    (4, 0, 4, 0), (4, 0, 1, 1), (4, 0, 2, 1),
    (4, 2, 4, 0), (4, 2, 1, 1), (4, 2, 2, 1),
    (5, 1, 5, 0), (5, 1, 0, 1), (5, 1, 2, 1),
    (5, 2, 5, 0), (5, 2, 0, 1), (5, 2, 2, 1),
    (6, 2, 6, 0), (7, 2, 7, 0), (7, 2, 2, 1),
]

# Pair slots in the p16 tile, ordered so every q-group's pair set is a
# contiguous slot range (enables one batched subtract per q-group):
_PAIR_SLOTS = [
    (0, 4), (0, 6), (2, 4), (4, 0), (4, 2),      # X1: q-triple {g4, w1, w2}
    (1, 5), (1, 6), (2, 5), (5, 1), (5, 2),      # X2: q-triple {g5, w0, w2}
    (2, 7), (7, 2), (2, 6), (6, 2),              # tail
]
# (q_key, pair_slot_range, d_slot_start): D[d0:d0+n] = P[p0:p1] - q
# q_key: (gamma_idx, is_not)
_QGROUPS = [
    ((4, 0), 0, 5, 0),     # g4:  D[0:5]   = P[0:5]  - q_g4
    ((1, 1), 0, 5, 5),     # w1:  D[5:10]  = P[0:5]  - q_w1
    ((2, 1), 0, 12, 10),   # w2:  D[10:22] = P[0:12] - q_w2
    ((5, 0), 5, 10, 22),   # g5:  D[22:27] = P[5:10] - q_g5
    ((0, 1), 5, 10, 27),   # w0:  D[27:32] = P[5:10] - q_w0
    ((7, 0), 10, 12, 32),  # g7:  D[32:34] = P[10:12]- q_g7
    ((6, 0), 12, 14, 34),  # g6:  D[34:36] = P[12:14]- q_g6
]
# Emission order of the d36 sub groups: the ACT-relu'd slot range (tail
# slots) first, so ACT can start its relu share while DVE finishes the rest.
_QGROUP_ORDER = [3, 4, 5, 6, 0, 1, 2]

# Engine-balance knobs (tuned against the CoreSim/HW traces). Env overrides
# let the bench script bisect without editing the file.
import os as _os
_POOL_PAIRS = int(_os.environ.get("K_POOL_PAIRS", "2"))
_POOL_GTREE = _os.environ.get("K_POOL_GTREE", "1") == "1"
_POOL_QP = _os.environ.get("K_POOL_QP", "1") == "1"
_POOL_TAIL = _os.environ.get("K_POOL_TAIL", "1") == "1"
_POOL_SMALL = _os.environ.get("K_POOL_SMALL", "1") == "1"  # sab/Sh/wp/qn
_ACT_RELU_SLOTS = int(_os.environ.get("K_ACT_RELU", "12"))


def _bcast_mid(ap, n):
    """[P, R] access pattern -> [P, n, R] with a zero-stride middle dim."""
    a = ap[:, :]
    return bass.AP(tensor=a.tensor, offset=a.offset,
                   ap=[a.ap[0], [0, n], a.ap[1]])


def _swap_free(ap):
    """View a [P, K, R] tile iterated as [P, R, K] (same memory)."""
    a = ap[:, :, :]
    return bass.AP(tensor=a.tensor, offset=a.offset,
                   ap=[a.ap[0], a.ap[2], a.ap[1]])


def build_kernel(reps: int = 1, rows_per_part: int = ROWS_PER_PART) -> bass.Bass:
    nc = bacc.Bacc("TRN2", target_bir_lowering=False, debug=False,
                   num_devices=N_CORES)

    # Restrict the ACT table chooser to the one set holding Exp+Ln (avoids
    # ~2.7us table reloads between Exp and Ln ops).
    _orig_tables = bacc.get_activation_tables

    def _one_set(arch):
        return {
            name: (fns if name == "natural_log_exp_and_others" else set())
            for name, fns in _orig_tables(arch).items()
        }

    # Register the exp-bias constant (-ln8) as a per-partition const AP the
    # same way Bass.__init__ registers 0.0/1.0 (activation converts float
    # biases to const APs and asserts one exists).
    t = nc.alloc_sbuf_tensor("const-f32-negln8", [128, 1], F32)
    nc.gpsimd.memset(t.ap(), -LN8)
    nc.const_aps.aps[(F32, -LN8)] = t.ap()
    nc.all_engine_barrier()

    bacc.get_activation_tables = _one_set
    try:
        return _build_body(nc, reps, rows_per_part)
    finally:
        bacc.get_activation_tables = _orig_tables


def _build_body(nc, reps: int, rows_per_part: int) -> bass.Bass:
    b_core = P * rows_per_part
    n_tiles = rows_per_part // R

    a_d = nc.dram_tensor("alpha_logits", [b_core, 8], F32, kind="ExternalInput")
    b_d = nc.dram_tensor("beta_logits", [b_core, 8], F32, kind="ExternalInput")
    g_d = nc.dram_tensor("gamma_logits", [b_core, 8], F32, kind="ExternalInput")
    o_d = nc.dram_tensor("loss", [b_core], F32, kind="ExternalOutput")

    a_v = a_d[:].rearrange("(p n) k -> p n k", p=P)
    b_v = b_d[:].rearrange("(p n) k -> p n k", p=P)
    g_v = g_d[:].rearrange("(p n) k -> p n k", p=P)
    o_v = o_d[:].rearrange("(p n) -> p n", p=P)

    with TileContext(nc) as tc:
        import contextlib
        rep_loop = tc.For_i(0, reps, 1) if reps > 1 else contextlib.nullcontext()
        with (
            rep_loop,
            tc.tile_pool(name="io", bufs=2) as io,
            tc.tile_pool(name="e8", bufs=1) as e8p,
            tc.tile_pool(name="work", bufs=1) as work,
            tc.tile_pool(name="p16p", bufs=2) as p16p,
            tc.tile_pool(name="big", bufs=1) as big,
            tc.tile_pool(name="outp", bufs=1) as outp,
        ):
            for j in range(n_tiles):
                sl = slice(j * R, (j + 1) * R)

                a_t = io.tile([P, R, 8], F32, tag="a")
                b_t = io.tile([P, R, 8], F32, tag="b")
                g_t = io.tile([P, R, 8], F32, tag="g")
                nc.sync.dma_start(out=a_t, in_=a_v[:, sl, :])
                nc.sync.dma_start(out=b_t, in_=b_v[:, sl, :])
                nc.sync.dma_start(out=g_t, in_=g_v[:, sl, :])

                # --- exps, k-major fp16 [P, 8, R] (strided out is free on
                # ACT). a/b exps carry bias -ln8 so sab = sa*sb/64 stays in
                # fp16 range; lns of these tiles give alpha-ln8 / beta-ln8.
                ea8 = e8p.tile([P, 8, R], F16, tag="ea8")
                eb8 = e8p.tile([P, 8, R], F16, tag="eb8")
                eg8 = e8p.tile([P, 8, R], F16, tag="eg8")
                nc.scalar.activation(out=_swap_free(ea8), in_=a_t,
                                     func=AF.Exp, bias=-LN8)
                nc.scalar.activation(out=_swap_free(eb8), in_=b_t,
                                     func=AF.Exp, bias=-LN8)
                nc.scalar.activation(out=_swap_free(eg8), in_=g_t,
                                     func=AF.Exp)

                # --- softmax denominators via fp16 tree adds (2x).
                # s4/s2 scratch is shared across the three tensors (the tree
                # ops serialize on DVE anyway); s1 sums must coexist.
                # a/b trees share scratch (both DVE, serialized anyway); the
                # g tree can run on Pool in parallel so it gets its own.
                sums = {}
                for name, e_t in (("a", ea8), ("b", eb8), ("g", eg8)):
                    on_pool = _POOL_GTREE and name == "g"
                    eng = nc.gpsimd if on_pool else nc.vector
                    sfx = "g" if on_pool else "ab"
                    s4 = work.tile([P, 4, R], F16, tag=f"s4{sfx}")
                    eng.tensor_add(s4, e_t[:, 0:4, :], e_t[:, 4:8, :])
                    eng.tensor_add(s4[:, 0:2, :], s4[:, 0:2, :], s4[:, 2:4, :])
                    s1 = work.tile([P, R], F16, tag=f"s1{name}")
                    eng.tensor_add(s1, s4[:, 0, :], s4[:, 1, :])
                    sums[name] = s1
                sg_t = sums["g"]

                # --- k-major logits via Ln(exp) on ACT (enables 2x pairs).
                # a8 = alpha - ln8 (in place over ea8), b8 = beta - ln8,
                # gq = gamma[4:8] (in place over eg8[4:8], dead after g-tree).
                a8, b8 = ea8, eb8
                nc.scalar.activation(out=a8, in_=ea8, func=AF.Ln)
                nc.scalar.activation(out=b8, in_=eb8, func=AF.Ln)
                gq = eg8[:, 4:8, :]
                nc.scalar.activation(out=gq, in_=eg8[:, 4:8, :], func=AF.Ln)

                # --- Sh = lsg - Ln(sab') = S + ln64 (cancels pair offset) ---
                small_eng = nc.gpsimd if _POOL_SMALL else nc.vector
                sab = work.tile([P, R], F16, tag="sab")
                small_eng.tensor_mul(sab, sums["a"], sums["b"])
                lsab = work.tile([P, R], F16, tag="lsab")
                nc.scalar.activation(out=lsab, in_=sab, func=AF.Ln)
                lsg = work.tile([P, R], F16, tag="lsg")
                nc.scalar.activation(out=lsg, in_=sg_t, func=AF.Ln)
                sh_t = work.tile([P, R], F16, tag="Sh")
                small_eng.tensor_sub(sh_t, lsg, lsab)

                # --- q tiles: qp[k] = gamma[4+k] - Sh (pos),
                #              qn[k] = ln(sg - eg[k]) - Sh (not, k in 0..2)
                # wl and qn run in place over wp; qp in place over gq.
                wp = work.tile([P, 3, R], F16, tag="wp")
                small_eng.tensor_sub(wp, _bcast_mid(sg_t, 3), eg8[:, 0:3, :])
                nc.scalar.activation(out=wp, in_=wp, func=AF.Ln)
                qn = wp
                small_eng.tensor_sub(qn, wp, _bcast_mid(sh_t, 3))

                # --- 14 pair sums (fp16 2x); emitted before qp so DVE does
                # not stall waiting for the Pool/ACT Sh chain. A few go to
                # Pool for balance.
                p16 = p16p.tile([P, 14, R], F16, tag="p16")
                for i, (ai, bi) in enumerate(_PAIR_SLOTS):
                    eng = nc.gpsimd if i < _POOL_PAIRS else nc.vector
                    eng.tensor_add(p16[:, i, :], a8[:, ai, :], b8[:, bi, :])

                qp = gq
                qp_eng = nc.gpsimd if _POOL_QP else nc.vector
                qp_eng.tensor_sub(qp, gq, _bcast_mid(sh_t, 4))
                q = {(4 + k, 0): qp[:, k, :] for k in range(4)}
                q.update({(k, 1): qn[:, k, :] for k in range(3)})

                # --- d36 = p - q (7 batched subs, 2x), relu (split DVE 4x /
                # ACT, they run on disjoint slot ranges), tree sum ---
                d36 = big.tile([P, 36, R], F16, tag="d36")
                for gi in _QGROUP_ORDER:
                    qkey, p0, p1, d0 = _QGROUPS[gi]
                    n = p1 - p0
                    nc.vector.tensor_sub(
                        d36[:, d0:d0 + n, :], p16[:, p0:p1, :],
                        _bcast_mid(q[qkey], n),
                    )
                ns = 36 - _ACT_RELU_SLOTS
                if _ACT_RELU_SLOTS:
                    nc.scalar.activation(out=d36[:, ns:36, :],
                                         in_=d36[:, ns:36, :], func=AF.Relu)
                nc.vector.tensor_scalar_max(out=d36[:, 0:ns, :],
                                            in0=d36[:, 0:ns, :], scalar1=0.0)

                nc.vector.tensor_add(d36[:, 0:18, :], d36[:, 0:18, :],
                                     d36[:, 18:36, :])
                nc.vector.tensor_add(d36[:, 0:9, :], d36[:, 0:9, :],
                                     d36[:, 9:18, :])
                nc.vector.tensor_add(d36[:, 0, :], d36[:, 0, :], d36[:, 8, :])
                nc.vector.tensor_add(d36[:, 0:4, :], d36[:, 0:4, :],
                                     d36[:, 4:8, :])
                nc.vector.tensor_add(d36[:, 0:2, :], d36[:, 0:2, :],
                                     d36[:, 2:4, :])
                loss_t = outp.tile([P, R], F32, tag="loss")
                tail_eng = nc.gpsimd if _POOL_TAIL else nc.vector
                tail_eng.tensor_add(loss_t, d36[:, 0, :], d36[:, 1, :])
                nc.sync.dma_start(out=o_v[:, sl], in_=loss_t)

    nc.compile()
    return nc


_NC_CACHE = None


def _get_nc():
    global _NC_CACHE
    if _NC_CACHE is None:
        _NC_CACHE = build_kernel()
    return _NC_CACHE


def kernel(alpha_logits, beta_logits, gamma_logits, _trace=False):
    nc = _get_nc()
    in_maps = []
    for c in range(N_CORES):
        sl = slice(c * B_CORE, (c + 1) * B_CORE)
        in_maps.append({
            "alpha_logits": np.ascontiguousarray(alpha_logits[sl]),
            "beta_logits": np.ascontiguousarray(beta_logits[sl]),
            "gamma_logits": np.ascontiguousarray(gamma_logits[sl]),
        })
    res = run_bass_kernel_spmd(nc, in_maps, core_ids=list(range(N_CORES)),
                               trace=_trace)
    out = np.concatenate([r["loss"] for r in res.results])
    if _trace:
        kernel.last_result = res
    return out
